# revision 22
# baseline (speedup 1.0000x reference)
"""Trainium2 Bass kernel for nn_CrossAttention (B=2, Lq=Lkv=2048, E=1024, H=16, D=64).

Sharding: 2-way data-parallel over batch x 4-way tensor-parallel over
heads. Core c owns batch c//4 and heads [4*(c%4), 4*(c%4)+4) -- a
256-wide feature slice handled as two 128-wide head-pairs. The host
sums the 4 partial outputs per batch (the row-parallel Wo all-reduce).

Vs. the TP8 variant this halves every DMA stream (x in, partial out)
while keeping PE/ACT work identical.

Key structure (shared with the TP8 variant):
  - kv positions with mask==0 are removed on the HOST (exact math:
    softmax gives them probability 0). The kernel is specialized per
    max-valid-chunk count and cached; all cores run the same program,
    per-core mask bias covers the count difference between batches.
  - bf16 for x, weights, probs, V, ctx and the output partial.
  - V is produced directly in [k, d] layout by a flipped matmul; bv
    rides in via a K=1 ones-row matmul; a ones column in the V
    stationary accumulates the softmax denominator in PSUM row 0.
  - Attention is software-pipelined: exp(k) on ACT overlaps
    scores(k+1) + ctx(k) on PE; the remaining Q-tile projections are
    threaded one matmul per chunk into the attention PE stream.
  - Division by the denominator is deferred past the next query's
    first scores; x loads / ctx writebacks go through the Pool (SWDGE)
    queue, output DMAs alternate SP/Pool HWDGE queues.
"""

import sys

if "/opt/trn_rl_repo" not in sys.path:
    sys.path.insert(0, "/opt/trn_rl_repo")

import numpy as np
import ml_dtypes

import concourse.tile as tile
from concourse import bacc, mybir
from concourse.bass_utils import run_bass_kernel_spmd

F32 = mybir.dt.float32
F32R = mybir.dt.float32r
BF16 = mybir.dt.bfloat16
AF = mybir.ActivationFunctionType
BF16NP = ml_dtypes.bfloat16

N_CORES = 8
B, LQ, LKV, E, H, D = 2, 2048, 2048, 1024, 16, 64
DP = 2  # data-parallel groups (batch)
TP = N_CORES // DP  # tensor-parallel cores per batch = 4
HC = H // TP  # heads per core = 4
NP = 2  # head pairs per core
JC = HC * D  # feature slice per core = 256
Tb = LQ  # tokens per core = its batch only
NEC = E // 128  # 8 e-chunks
NQT = Tb // 512  # 4 q tiles
NVQ = NP * NQT  # 8 virtual queries (pair, qtile)
NOC = E // 128  # 8 output chunks

_NC_CACHE = {}
_LAST_META = None


def build(reps=None, phases="PAO", meta=None):
    """meta = nktm: max number of 128-wide valid kv chunks over batches."""
    global _LAST_META
    if meta is None:
        meta = _LAST_META
    assert meta is not None, "call make_in_maps first (sets kv chunk count)"
    nktm = meta
    key = (reps or 0, phases, meta)
    if key in _NC_CACHE:
        return _NC_CACHE[key]
    nkv5 = -(-nktm * 128 // 512)  # 512-wide projection tiles
    kvw = nkv5 * 512
    NG = nktm

    nc = bacc.Bacc("TRN2", target_bir_lowering=False, debug=False, num_devices=N_CORES)

    xqT = nc.dram_tensor("xqT", [E, Tb], BF16, kind="ExternalInput").ap()
    xkT = nc.dram_tensor("xkT", [E, kvw], BF16, kind="ExternalInput").ap()
    # packed constants: wq|wk|wv (each [128, 8*256]) | wo [128, 2*8*128] + bv row
    cbd = nc.dram_tensor("cb", [128, 8 * E + JC], BF16, kind="ExternalInput").ap()
    # packed fp32 constants: bq[2] | bk[2] | mask-bias chunks
    cfd = nc.dram_tensor("cf", [128, 4 + NG], F32, kind="ExternalInput").ap()
    outT = nc.dram_tensor("outT", [E, Tb], BF16, kind="ExternalOutput").ap()

    from contextlib import nullcontext

    with tile.TileContext(nc) as tc, nc.allow_low_precision(reason="bf16 kernel"):
        with tc.For_i(0, reps, 1) if reps else nullcontext():
         with (
             tc.tile_pool(name="const", bufs=1) as const,
             tc.tile_pool(name="big", bufs=1) as big,
         ):
             # ---- persistent SBUF state ----
             cb_sb = const.tile([128, 8 * E + JC], BF16, tag="cb")
             nc.sync.dma_start(out=cb_sb, in_=cbd)
             cf_sb = const.tile([128, 4 + NG], F32, tag="cf")
             nc.sync.dma_start(out=cf_sb, in_=cfd)
             W2 = 2 * E  # 2048 per weight block
             wq_sb = cb_sb[:, 0 * W2 : 1 * W2].rearrange("p (ec j) -> p ec j", ec=NEC)
             wk_sb = cb_sb[:, 1 * W2 : 2 * W2].rearrange("p (ec j) -> p ec j", ec=NEC)
             wv_sb = cb_sb[:, 2 * W2 : 3 * W2].rearrange("p (ec j) -> p ec j", ec=NEC)
             wo_sb = cb_sb[:, 3 * W2 : 4 * W2].rearrange(
                 "p (jh oc o) -> p jh oc o", jh=NP, oc=NOC
             )
             bv_sb = cb_sb[0:1, 4 * W2 : 4 * W2 + JC]
             bq_sb = cf_sb[:, 0:2]
             bk_sb = cf_sb[:, 2:4]
             mb_sb = cf_sb[:, 4:]
             ones1 = const.tile([1, 128], BF16, tag="ones1")
             nc.vector.memset(ones1, 1.0)
             ones_f = const.tile([1, 65], F32, tag="onesf")
             nc.vector.memset(ones_f, 1.0)
             onesc = const.tile([1, 65], F32R, tag="onesc")
             nc.vector.tensor_copy(onesc, ones_f)

             qt_sb = big.tile([128, NP, Tb], BF16, tag="qt")
             kt_sb = big.tile([128, NP, kvw], BF16, tag="kt")
             xq_sb = big.tile([128, NQT, NEC, 512], BF16, tag="xq")
             # V per (pair, chunk): [1 | h0 d0..63 | 1 | h1 d0..63]
             v_sb = big.tile([128, NP, NG, 130], BF16, tag="v")
             nc.vector.memset(v_sb[:, :, :, 0:1], 1.0)
             nc.vector.memset(v_sb[:, :, :, 65:66], 1.0)
             ctx_sb = big.tile([128, NP, NQT, 512], BF16, tag="ctx")

             # ---- phase P: kv projections + first Q tiles ----
             if "P" in phases:
              with (
                 tc.tile_pool(name="xin", bufs=3) as xin,
                 tc.tile_pool(name="pp", bufs=2, space="PSUM") as pp,
                 tc.tile_pool(name="vp", bufs=4, space="PSUM") as vp,
             ):
                 for t5 in range(nkv5):
                     toff = t5 * 512
                     xt = xin.tile([128, NEC, 512], BF16, tag="xin")
                     nc.gpsimd.dma_start(
                         out=xt,
                         in_=xkT[:, toff : toff + 512].rearrange(
                             "(ec p) t -> p ec t", p=128
                         ),
                     )
                     for hp in range(NP):
                         pt = pp.tile([128, 512], F32, tag="pp")
                         for ec in range(NEC):
                             nc.tensor.matmul(
                                 pt,
                                 wk_sb[:, ec, hp * 128 : (hp + 1) * 128],
                                 xt[:, ec, :],
                                 start=(ec == 0),
                                 stop=(ec == NEC - 1),
                             )
                         nc.scalar.activation(
                             out=kt_sb[:, hp, toff : toff + 512],
                             in_=pt,
                             func=AF.Identity,
                             bias=bk_sb[:, hp : hp + 1],
                             scale=1.0,
                         )
                     # V chunks within this 512-token tile, [k, d] layout
                     for ck in range(4):
                         gc = t5 * 4 + ck
                         if gc >= nktm:
                             break
                         vt = vp.tile([128, NP, 128], F32, tag="vp")
                         vflat = vt.rearrange("p a d -> p (a d)")
                         for ec in range(NEC):
                             nc.tensor.matmul(
                                 vflat,
                                 xt[:, ec, ck * 128 : (ck + 1) * 128],
                                 wv_sb[:, ec, :],
                                 start=(ec == 0),
                                 stop=False,
                             )
                         nc.tensor.matmul(vflat, ones1, bv_sb, start=False, stop=True)
                         for hp in range(NP):
                             # cols (1:65, 66:130) <- PSUM pair cols 0:128
                             nc.vector.tensor_copy(
                                 v_sb[:, hp, gc, :].rearrange(
                                     "p (a d) -> p a d", d=65
                                 )[:, :, 1:65],
                                 vt[:, hp, :].rearrange("p (a d) -> p a d", a=2),
                             )

                 for tt in range(NQT):
                     nc.gpsimd.dma_start(
                         out=xq_sb[:, tt],
                         in_=xqT[:, tt * 512 : (tt + 1) * 512].rearrange(
                             "(ec p) t -> p ec t", p=128
                         ),
                     )
                 for vq in range(2):  # virtual queries 0,1 = (pair 0, tiles 0-1)
                     hp, tt = divmod(vq, NQT)
                     pt = pp.tile([128, 512], F32, tag="pp")
                     for ec in range(NEC):
                         nc.tensor.matmul(
                             pt,
                             wq_sb[:, ec, hp * 128 : (hp + 1) * 128],
                             xq_sb[:, tt, ec, :],
                             start=(ec == 0),
                             stop=(ec == NEC - 1),
                         )
                     nc.scalar.activation(
                         out=qt_sb[:, hp, tt * 512 : (tt + 1) * 512],
                         in_=pt,
                         func=AF.Identity,
                         bias=bq_sb[:, hp : hp + 1],
                         scale=1.0,
                     )

             # ---- phase A: attention (software-pipelined) ----
             if "A" in phases:
              with (
                 tc.tile_pool(name="attps", bufs=2, space="PSUM") as attps,
                 tc.tile_pool(name="cxps", bufs=1, space="PSUM") as cxps,
                 tc.tile_pool(name="qpp", bufs=1, space="PSUM") as qpp,
                 tc.tile_pool(name="expm", bufs=3) as expm,
                 tc.tile_pool(name="dv", bufs=4) as dv,
             ):

                 def scores(hp, q0, kt):
                     k0 = kt * 128
                     sct = attps.tile([128, 2, 512], F32, tag="sc")
                     for h in range(2):
                         nc.tensor.matmul(
                             sct[:, h, :],
                             kt_sb[h * 64 : (h + 1) * 64, hp, k0 : k0 + 128],
                             qt_sb[h * 64 : (h + 1) * 64, hp, q0 : q0 + 512],
                             start=True,
                             stop=True,
                         )
                     return sct

                 def div_flush(pend):
                     # bct matmul + normalize + ctx writeback for a finished
                     # query tile; emitted after the next tile's first scores
                     # so the PE keeps streaming.
                     cxs, rrs, s2s, hp, qt = pend
                     for h in range(2):
                         bct = cxps.tile([65, 512], F32, tag="bct")
                         nc.tensor.matmul(bct, onesc, rrs[h], start=True, stop=True)
                         cs = dv.tile([65, 512], BF16, tag="cs")
                         nc.vector.tensor_mul(cs, s2s[h], bct)
                         nc.gpsimd.dma_start(
                             out=ctx_sb[h * 64 : (h + 1) * 64, hp, qt, :],
                             in_=cs[1:65, :],
                         )

                 pend = None
                 qproj = []  # deferred matmuls of the in-flight Q projection
                 for hp in range(NP):
                     for qt in range(NQT):
                         q0 = qt * 512
                         vq = hp * NQT + qt
                         # set up interleaved projection of virtual query vq+2
                         if vq + 2 < NVQ:
                             hp2, tt2 = divmod(vq + 2, NQT)
                             qp = qpp.tile([128, 512], F32, tag="qp", name=f"qp{vq+2}")

                             def qp_mm(ec, qp=qp, hp2=hp2, tt2=tt2):
                                 nc.tensor.matmul(
                                     qp,
                                     wq_sb[:, ec, hp2 * 128 : (hp2 + 1) * 128],
                                     xq_sb[:, tt2, ec, :],
                                     start=(ec == 0),
                                     stop=(ec == NEC - 1),
                                 )

                             qproj = [(qp_mm, ec) for ec in range(NEC)]
                             qp_fin = (qp, hp2, tt2)
                         cxs = [
                             cxps.tile([65, 512], F32, tag=f"cx{h}", name=f"cx{h}_{vq}")
                             for h in range(2)
                         ]
                         sct = scores(hp, q0, 0)
                         if pend is not None:
                             div_flush(pend)
                             pend = None
                         for kt in range(nktm):
                             emt = expm.tile([128, 2, 512], BF16, tag="expm")
                             nc.scalar.activation(
                                 out=emt.rearrange("p a t -> p (a t)"),
                                 in_=sct.rearrange("p a t -> p (a t)"),
                                 func=AF.Exp,
                                 bias=mb_sb[:, kt : kt + 1],
                                 scale=0.125,
                             )
                             if kt + 1 < nktm:
                                 sct = scores(hp, q0, kt + 1)
                             if qproj:
                                 fn, ec = qproj.pop(0)
                                 fn(ec)
                             st, sp = (kt == 0), (kt == nktm - 1)
                             for h in range(2):
                                 nc.tensor.matmul(
                                     cxs[h],
                                     v_sb[:, hp, kt, h * 65 : (h + 1) * 65],
                                     emt[:, h, :],
                                     start=st,
                                     stop=sp,
                                 )
                         for fn, ec in qproj:  # leftovers (sparse masks)
                             fn(ec)
                         qproj = []
                         if vq + 2 < NVQ:
                             qp, hp2, tt2 = qp_fin
                             nc.vector.tensor_scalar_add(
                                 qt_sb[:, hp2, tt2 * 512 : (tt2 + 1) * 512],
                                 qp,
                                 bq_sb[:, hp2 : hp2 + 1],
                             )
                         rrs, s2s = [], []
                         for h in range(2):
                             rr = dv.tile([1, 512], F32R, tag="rr")
                             nc.vector.reciprocal(rr, cxs[h][0:1, :])
                             rrs.append(rr)
                             s2 = dv.tile([65, 512], F32R, tag=f"s2{h}")
                             nc.vector.tensor_copy(s2, cxs[h])
                             s2s.append(s2)
                         pend = (cxs, rrs, s2s, hp, qt)
                 div_flush(pend)

             # ---- phase O: output projection (partial; host sums cores) ----
             # oc-major: one 256KB DMA per output row-chunk, alternating
             # SP / Pool queues; copies alternate ACT / DVE.
             if "O" in phases:
              with (
                 tc.tile_pool(name="ops", bufs=3, space="PSUM") as ops,
                 tc.tile_pool(name="outsb", bufs=2) as outsb,
             ):
                 for oc in range(NOC):
                     ob = outsb.tile([128, NQT, 512], BF16, tag="ob")
                     for tt2 in range(NQT // 2):
                         opt = ops.tile([128, 2, 512], F32, tag="op")
                         for half in range(2):
                             tt = tt2 * 2 + half
                             for jh in range(NP):
                                 nc.tensor.matmul(
                                     opt[:, half, :],
                                     wo_sb[:, jh, oc, :],
                                     ctx_sb[:, jh, tt, :],
                                     start=(jh == 0),
                                     stop=(jh == NP - 1),
                                 )
                         obp = ob[:, tt2 * 2 : tt2 * 2 + 2, :]
                         if tt2 % 2 == 0:
                             nc.scalar.activation(
                                 out=obp.rearrange("p a t -> p (a t)"),
                                 in_=opt.rearrange("p a t -> p (a t)"),
                                 func=AF.Identity,
                                 scale=1.0,
                             )
                         else:
                             nc.vector.tensor_copy(obp, opt)
                     eng = nc.sync if oc % 2 == 0 else nc.gpsimd
                     eng.dma_start(
                         out=outT[oc * 128 : (oc + 1) * 128, :],
                         in_=ob.rearrange("p a t -> p (a t)"),
                     )

    nc.compile()
    _NC_CACHE[key] = nc
    return nc


def make_in_maps(query, key_value, mask, Wq, bq, Wk, bk, Wv, bv, Wo, bo):
    global _LAST_META
    # pack valid kv positions per batch (mask==0 rows contribute exactly 0)
    idx = [np.nonzero(mask[b] != 0)[0] for b in range(B)]
    cnt = [len(i) for i in idx]
    nkt = [-(-c // 128) for c in cnt]  # valid 128-chunks per batch
    nktm = max(max(nkt), 1)
    _LAST_META = nktm
    nkv5 = -(-nktm * 128 // 512)
    kvw = nkv5 * 512
    NG = nktm

    xq = [
        np.ascontiguousarray(query[b].reshape(Tb, E).T).astype(BF16NP)
        for b in range(B)
    ]
    xk, mbs = [], []
    for b in range(B):
        xb = np.zeros((kvw, E), np.float32)
        xb[: cnt[b]] = key_value[b][idx[b]]
        xk.append(np.ascontiguousarray(xb.T).astype(BF16NP))
        mb = np.full((128, NG), -1.0e5, np.float32)
        for k in range(nkt[b]):
            valid = min(cnt[b] - k * 128, 128)
            mb[:valid, k] = 0.0
        mbs.append(mb)

    def wslice(W, sl):
        # W[sl,:].T laid out [p, (ec j)]: wT[(ec p), j] -> [p, ec, j]
        wT = np.ascontiguousarray(W[sl, :].T)
        return wT.reshape(NEC, 128, JC).transpose(1, 0, 2).reshape(128, 2 * E)

    in_maps = []
    for c in range(N_CORES):
        bc, hq = divmod(c, TP)
        sl = slice(hq * JC, (hq + 1) * JC)
        W2 = 2 * E
        cb = np.zeros((128, 8 * E + JC), np.float32)
        cb[:, 0 * W2 : 1 * W2] = wslice(Wq, sl)
        cb[:, 1 * W2 : 2 * W2] = wslice(Wk, sl)
        cb[:, 2 * W2 : 3 * W2] = wslice(Wv, sl)
        # wo: [256, 1024] -> [p, (jh oc o)]
        woT = np.ascontiguousarray(Wo[:, sl].T)
        cb[:, 3 * W2 : 4 * W2] = (
            woT.reshape(NP, 128, E).transpose(1, 0, 2).reshape(128, 2 * E)
        )
        cb[0, 4 * W2 : 4 * W2 + JC] = bv[sl]
        cf = np.zeros((128, 4 + NG), np.float32)
        cf[:, 0:2] = bq[sl].reshape(NP, 128).T
        cf[:, 2:4] = bk[sl].reshape(NP, 128).T
        cf[:, 4:] = mbs[bc]
        in_maps.append(
            {
                "xqT": xq[bc],
                "xkT": xk[bc],
                "cb": cb.astype(BF16NP),
                "cf": cf,
            }
        )
    return in_maps


def kernel(query, key_value, mask, Wq, bq, Wk, bk, Wv, bv, Wo, bo):
    in_maps = make_in_maps(
        np.asarray(query), np.asarray(key_value), np.asarray(mask),
        np.asarray(Wq), np.asarray(bq), np.asarray(Wk), np.asarray(bk),
        np.asarray(Wv), np.asarray(bv), np.asarray(Wo), np.asarray(bo),
    )
    nc = build()
    res = run_bass_kernel_spmd(nc, in_maps, list(range(N_CORES)))
    out = np.zeros((B, Tb, E), np.float32)
    for c in range(N_CORES):
        bc = c // TP
        out[bc] += res.results[c]["outT"].astype(np.float32).T
    out += np.asarray(bo, np.float32)[None, None, :]
    return out.astype(np.float32)
